# revision 11
# baseline (speedup 1.0000x reference)
"""Camembert self-attention on 8 Trainium2 NeuronCores.

B=4, S=2048, H=1024, NH=16, HD=64. Sharding: core k handles batch k//2 and
head-group k%2 (8 heads = 512 output dims). Each core:
  xT = x.T (PE transpose), qT/kT = (x@W).T, v = x@W   (float32r matmuls)
  scoresT[tk,tq] = kT.T@qT per head (2 heads row-packed in the PE array)
  expT = exp(SCALE*scoresT)  (ACT, straight from PSUM)
  ctxT+ = [v|1].T @ expT  -> [65, Tq]  (row 64 = softmax denominators)
Host divides by row 64, transposes, and reassembles the full output.
"""

import sys

sys.path.insert(0, "/opt/trn_rl_repo")

import numpy as np

import concourse.bass as bass  # noqa: F401  (registers AP machinery)
import concourse.tile as tile
from concourse import bacc, mybir
from concourse.bass_utils import run_bass_kernel_spmd
from contextlib import ExitStack

P = 128
T = 2048          # tokens per core (one batch)
H = 1024          # hidden
D = 512           # output dims per core (8 heads x 64)
HD = 64
NHL = 8           # heads per core
HC = H // P       # 8 contraction chunks
TT = T // P       # 16 token tiles
DO = D // P       # 4
TKC = T // P      # 16 key chunks
SCALE = 0.125
F32 = mybir.dt.float32
F32R = mybir.dt.float32r

_CACHE = {}


def _emit(tc, x, wq, wk, wv, aux, out):
    nc = tc.nc
    Exp = mybir.ActivationFunctionType.Exp

    with ExitStack() as ctx:
        const = ctx.enter_context(tc.tile_pool(name="const", bufs=1))
        ident = const.tile([P, P], F32R)
        nc.sync.dma_start(ident[:], aux[:])

        qkv = ctx.enter_context(tc.tile_pool(name="qkv", bufs=1))
        qT = qkv.tile([P, DO, T], F32R, tag="qT")
        kT = qkv.tile([P, DO, T], F32R, tag="kT")
        vS = qkv.tile([P, TT, NHL * 65], F32R, tag="v")

        psA = ctx.enter_context(tc.tile_pool(name="psA", bufs=2, space="PSUM"))
        psB = ctx.enter_context(tc.tile_pool(name="psB", bufs=4, space="PSUM"))

        # ---- phase 0+1: x transpose + projections (pools closed after) ----
        with ExitStack() as s01:
            xlp = s01.enter_context(tc.tile_pool(name="xl", bufs=2))
            xTp = s01.enter_context(tc.tile_pool(name="xT", bufs=1))
            wp = s01.enter_context(tc.tile_pool(name="w", bufs=2))
            wvp = s01.enter_context(tc.tile_pool(name="wv", bufs=1))

            xT = xTp.tile([P, HC, T], F32R)
            for tt in range(TT):
                xt = xlp.tile([P, H], F32R, tag="x")
                nc.sync.dma_start(xt[:], x[tt * P:(tt + 1) * P, :])
                for hc in range(HC):
                    pt = psB.tile([P, P], F32R, tag="acc")
                    nc.tensor.transpose(pt[:], xt[:, hc * P:(hc + 1) * P], ident[:])
                    nc.any.tensor_copy(xT[:, hc, tt * P:(tt + 1) * P], pt[:])

            # q/k projections -> transposed layout [dp, do, t]
            for wdram, dstT in ((wq, qT), (wk, kT)):
                wr = wdram.rearrange("(hc p) d -> p hc d", p=P)
                for do in range(DO):
                    wt = wp.tile([P, HC, P], F32R, tag="w")
                    nc.sync.dma_start(wt[:], wr[:, :, do * P:(do + 1) * P])
                    for t4 in range(T // 512):
                        ps = psB.tile([P, 512], F32, tag="acc")
                        for hc in range(HC):
                            nc.tensor.matmul(
                                ps[:],
                                lhsT=wt[:, hc, :],
                                rhs=xT[:, hc, t4 * 512:(t4 + 1) * 512],
                                start=(hc == 0),
                                stop=(hc == HC - 1),
                            )
                        nc.any.tensor_copy(
                            dstT[:, do, t4 * 512:(t4 + 1) * 512], ps[:])

            # v projection -> natural layout, interleaved into vS (65-stride)
            wvr = wv.rearrange("(hc p) d -> p hc d", p=P)
            wvt = wvp.tile([P, HC, D], F32R, tag="wv")
            nc.sync.dma_start(wvt[:], wvr[:])
            for ttb in range(TT // 4):
                for tt in range(ttb * 4, ttb * 4 + 4):
                    ps = psB.tile([P, 512], F32, tag="acc")
                    for hc in range(HC):
                        nc.tensor.matmul(
                            ps[:],
                            lhsT=xT[:, hc, tt * P:(tt + 1) * P],
                            rhs=wvt[:, hc, :],
                            start=(hc == 0),
                            stop=(hc == HC - 1),
                        )
                    nc.any.tensor_copy(
                        vS[:, tt].rearrange("p (h e) -> p h e", e=65)[:, :, 0:64],
                        ps[:].rearrange("p (h e) -> p h e", e=64),
                    )
                    # denominator column: 1.0 at col 64 of each head block
                    nc.vector.tensor_scalar(
                        vS[:, tt].rearrange("p (h e) -> p h e", e=65)[:, :, 64:65],
                        ps[:, 0:NHL].rearrange("p (h o) -> p h o", o=1),
                        0.0,
                        1.0,
                        mybir.AluOpType.mult,
                        mybir.AluOpType.add,
                    )

        # ---- phase 2: attention ----
        ep = ctx.enter_context(tc.tile_pool(name="e", bufs=4))
        for j in range(NHL // 2):     # head pairs (partitions 0:64 / 64:128)
            hA, hB = 2 * j, 2 * j + 1
            for tq in range(T // 1024):
                cps = [psB.tile([65, 512], F32, tag="acc", name=f"cp{i}")
                       for i in range(4)]
                for c in range(TKC):
                    sA = psA.tile([P, 1024], F32, tag="s")
                    sB = psA.tile([P, 1024], F32, tag="s")
                    for hq in range(2):
                        t0 = tq * 1024 + hq * 512
                        for s, lo in ((sA, 0), (sB, 64)):
                            nc.tensor.matmul(
                                s[:, hq * 512:(hq + 1) * 512],
                                lhsT=kT[lo:lo + 64, j, c * P:(c + 1) * P],
                                rhs=qT[lo:lo + 64, j, t0:t0 + 512],
                                start=True,
                                stop=True,
                            )
                    eA = ep.tile([P, 1024], F32R, tag="e")
                    eB = ep.tile([P, 1024], F32R, tag="e")
                    nc.scalar.activation(eA[:], sA[:], Exp, scale=SCALE)
                    nc.scalar.activation(eB[:], sB[:], Exp, scale=SCALE)
                    for hq in range(2):
                        for i, (h, e) in enumerate(((hA, eA), (hB, eB))):
                            nc.tensor.matmul(
                                cps[2 * i + hq][:],
                                lhsT=vS[:, c, h * 65:h * 65 + 65],
                                rhs=e[:, hq * 512:(hq + 1) * 512],
                                start=(c == 0),
                                stop=(c == TKC - 1),
                            )
                for i, h in enumerate((hA, hA, hB, hB)):
                    hq = i % 2
                    t0 = tq * 1024 + hq * 512
                    ot = ep.tile([65, 512], F32, tag="o", name=f"ot{i}")
                    nc.any.tensor_copy(ot[:], cps[2 * (i // 2) + hq][:])
                    nc.sync.dma_start(out[h, :, t0:t0 + 512], ot[:])


def _build():
    nc = bacc.Bacc(
        "TRN2",
        target_bir_lowering=False,
        debug=False,
        enable_asserts=False,
        num_devices=8,
    )
    x = nc.dram_tensor("x", [T, H], F32R, kind="ExternalInput").ap()
    wq = nc.dram_tensor("wq", [H, D], F32R, kind="ExternalInput").ap()
    wk = nc.dram_tensor("wk", [H, D], F32R, kind="ExternalInput").ap()
    wv = nc.dram_tensor("wv", [H, D], F32R, kind="ExternalInput").ap()
    aux = nc.dram_tensor("aux", [P, P], F32R, kind="ExternalInput").ap()
    out = nc.dram_tensor("out", [NHL, 65, T], F32, kind="ExternalOutput").ap()
    with tile.TileContext(nc) as tc:
        _emit(tc, x, wq, wk, wv, aux, out)
    nc.compile()
    return nc


def _get_nc():
    if "nc" not in _CACHE:
        _CACHE["nc"] = _build()
    return _CACHE["nc"]


def kernel(hidden_states, Wq, bq, Wk, bk, Wv, bv, **_):
    hidden_states = np.asarray(hidden_states, dtype=np.float32)
    Wq = np.asarray(Wq, dtype=np.float32)
    Wk = np.asarray(Wk, dtype=np.float32)
    Wv = np.asarray(Wv, dtype=np.float32)
    B, S, Hf = hidden_states.shape

    nc = _get_nc()
    in_maps = []
    for k in range(8):
        b, g = k // 2, k % 2
        sl = slice(g * D, (g + 1) * D)
        in_maps.append({
            "x": np.ascontiguousarray(hidden_states[b]),
            "wq": np.ascontiguousarray(Wq[:, sl]),
            "wk": np.ascontiguousarray(Wk[:, sl]),
            "wv": np.ascontiguousarray(Wv[:, sl]),
            "aux": np.eye(P, dtype=np.float32),
        })
    res = run_bass_kernel_spmd(nc, in_maps, core_ids=list(range(8)))

    outf = np.empty((B, S, Hf), dtype=np.float32)
    for k in range(8):
        b, g = k // 2, k % 2
        r = res.results[k]["out"]                  # [8, 65, 2048]
        ctx = r[:, :64, :] / r[:, 64:65, :]        # [8, 64, 2048]
        outf[b, :, g * D:(g + 1) * D] = (
            ctx.transpose(2, 0, 1).reshape(T, D))
    return outf


# revision 13
# speedup vs baseline: 1.5011x; 1.5011x over previous
"""Camembert self-attention on 8 Trainium2 NeuronCores.

B=4, S=2048, H=1024, NH=16, HD=64. Sharding: core k handles batch k//2 and
head-group k%2 (8 heads = 512 output dims). Each core:
  xT = x.T (PE transpose), qT/kT = (x@W).T, v = x@W   (float32r matmuls)
  scoresT[tk,tq] = kT.T@qT per head (2 heads row-packed in the PE array)
  expT = exp(SCALE*scoresT)  (ACT, straight from PSUM)
  ctxT+ = [v|1].T @ expT  -> [65, Tq]  (row 64 = softmax denominators)
Host divides by row 64, transposes, and reassembles the full output.
"""

import sys

sys.path.insert(0, "/opt/trn_rl_repo")

import numpy as np
import ml_dtypes

import concourse.bass as bass  # noqa: F401  (registers AP machinery)
import concourse.tile as tile
from concourse import bacc, mybir
from concourse.bass_utils import run_bass_kernel_spmd
from contextlib import ExitStack

P = 128
T = 2048          # tokens per core (one batch)
H = 1024          # hidden
D = 512           # output dims per core (8 heads x 64)
HD = 64
NHL = 8           # heads per core
HC = H // P       # 8 contraction chunks
TT = T // P       # 16 token tiles
DO = D // P       # 4
TKC = T // P      # 16 key chunks
SCALE = 0.125
F32 = mybir.dt.float32
F32R = mybir.dt.float32r
BF16 = mybir.dt.bfloat16
MM_DT = BF16          # dtype for matmul operands (BF16 or F32R)

_CACHE = {}


def _emit(tc, x, wq, wk, wv, aux, out):
    nc = tc.nc
    Exp = mybir.ActivationFunctionType.Exp

    with ExitStack() as ctx:
        const = ctx.enter_context(tc.tile_pool(name="const", bufs=1))
        ident = const.tile([P, P], MM_DT)
        nc.sync.dma_start(ident[:], aux[:])

        qkv = ctx.enter_context(tc.tile_pool(name="qkv", bufs=1))
        qT = qkv.tile([P, DO, T], MM_DT, tag="qT")
        kT = qkv.tile([P, DO, T], MM_DT, tag="kT")
        vS = qkv.tile([P, TT, NHL * 65], MM_DT, tag="v")

        psA = ctx.enter_context(tc.tile_pool(name="psA", bufs=2, space="PSUM"))
        psB = ctx.enter_context(tc.tile_pool(name="psB", bufs=4, space="PSUM"))

        # ---- phase 0+1: x transpose + projections (pools closed after) ----
        with ExitStack() as s01:
            xlp = s01.enter_context(tc.tile_pool(name="xl", bufs=2))
            xTp = s01.enter_context(tc.tile_pool(name="xT", bufs=1))
            wp = s01.enter_context(tc.tile_pool(name="w", bufs=2))
            wvp = s01.enter_context(tc.tile_pool(name="wv", bufs=1))

            xT = xTp.tile([P, HC, T], MM_DT)
            for tt in range(TT):
                xt = xlp.tile([P, H], MM_DT, tag="x")
                nc.sync.dma_start(xt[:], x[tt * P:(tt + 1) * P, :])
                for hc in range(HC):
                    pt = psB.tile([P, P], MM_DT, tag="acc")
                    nc.tensor.transpose(pt[:], xt[:, hc * P:(hc + 1) * P], ident[:])
                    nc.any.tensor_copy(xT[:, hc, tt * P:(tt + 1) * P], pt[:])

            # q/k projections -> transposed layout [dp, do, t]
            for wdram, dstT in ((wq, qT), (wk, kT)):
                wr = wdram.rearrange("(hc p) d -> p hc d", p=P)
                for do in range(DO):
                    wt = wp.tile([P, HC, P], MM_DT, tag="w")
                    nc.sync.dma_start(wt[:], wr[:, :, do * P:(do + 1) * P])
                    for t4 in range(T // 512):
                        ps = psB.tile([P, 512], F32, tag="acc")
                        for hc in range(HC):
                            nc.tensor.matmul(
                                ps[:],
                                lhsT=wt[:, hc, :],
                                rhs=xT[:, hc, t4 * 512:(t4 + 1) * 512],
                                start=(hc == 0),
                                stop=(hc == HC - 1),
                            )
                        nc.any.tensor_copy(
                            dstT[:, do, t4 * 512:(t4 + 1) * 512], ps[:])

            # v projection -> natural layout, interleaved into vS (65-stride)
            wvr = wv.rearrange("(hc p) d -> p hc d", p=P)
            wvt = wvp.tile([P, HC, D], MM_DT, tag="wv")
            nc.sync.dma_start(wvt[:], wvr[:])
            for ttb in range(TT // 4):
                for tt in range(ttb * 4, ttb * 4 + 4):
                    ps = psB.tile([P, 512], F32, tag="acc")
                    for hc in range(HC):
                        nc.tensor.matmul(
                            ps[:],
                            lhsT=xT[:, hc, tt * P:(tt + 1) * P],
                            rhs=wvt[:, hc, :],
                            start=(hc == 0),
                            stop=(hc == HC - 1),
                        )
                    nc.any.tensor_copy(
                        vS[:, tt].rearrange("p (h e) -> p h e", e=65)[:, :, 0:64],
                        ps[:].rearrange("p (h e) -> p h e", e=64),
                    )
                    # denominator column: 1.0 at col 64 of each head block
                    nc.vector.tensor_scalar(
                        vS[:, tt].rearrange("p (h e) -> p h e", e=65)[:, :, 64:65],
                        ps[:, 0:NHL].rearrange("p (h o) -> p h o", o=1),
                        0.0,
                        1.0,
                        mybir.AluOpType.mult,
                        mybir.AluOpType.add,
                    )

        # ---- phase 2: attention ----
        ep = ctx.enter_context(tc.tile_pool(name="e", bufs=4))
        for j in range(NHL // 2):     # head pairs (partitions 0:64 / 64:128)
            hA, hB = 2 * j, 2 * j + 1
            for tq in range(T // 1024):
                cps = [psB.tile([65, 512], F32, tag="acc", name=f"cp{i}")
                       for i in range(4)]
                for c in range(TKC):
                    sA = psA.tile([P, 1024], F32, tag="s")
                    sB = psA.tile([P, 1024], F32, tag="s")
                    for hq in range(2):
                        t0 = tq * 1024 + hq * 512
                        for s, lo in ((sA, 0), (sB, 64)):
                            nc.tensor.matmul(
                                s[:, hq * 512:(hq + 1) * 512],
                                lhsT=kT[lo:lo + 64, j, c * P:(c + 1) * P],
                                rhs=qT[lo:lo + 64, j, t0:t0 + 512],
                                start=True,
                                stop=True,
                            )
                    eA = ep.tile([P, 1024], MM_DT, tag="e")
                    eB = ep.tile([P, 1024], MM_DT, tag="e")
                    nc.scalar.activation(eA[:], sA[:], Exp, scale=SCALE)
                    nc.scalar.activation(eB[:], sB[:], Exp, scale=SCALE)
                    for hq in range(2):
                        for i, (h, e) in enumerate(((hA, eA), (hB, eB))):
                            nc.tensor.matmul(
                                cps[2 * i + hq][:],
                                lhsT=vS[:, c, h * 65:h * 65 + 65],
                                rhs=e[:, hq * 512:(hq + 1) * 512],
                                start=(c == 0),
                                stop=(c == TKC - 1),
                            )
                for i, h in enumerate((hA, hA, hB, hB)):
                    hq = i % 2
                    t0 = tq * 1024 + hq * 512
                    ot = ep.tile([65, 512], F32, tag="o", name=f"ot{i}")
                    nc.vector.tensor_copy(ot[:], cps[2 * (i // 2) + hq][:])
                    nc.sync.dma_start(out[h, :, t0:t0 + 512], ot[:])


def _build():
    nc = bacc.Bacc(
        "TRN2",
        target_bir_lowering=False,
        debug=False,
        enable_asserts=False,
        num_devices=8,
    )
    x = nc.dram_tensor("x", [T, H], MM_DT, kind="ExternalInput").ap()
    wq = nc.dram_tensor("wq", [H, D], MM_DT, kind="ExternalInput").ap()
    wk = nc.dram_tensor("wk", [H, D], MM_DT, kind="ExternalInput").ap()
    wv = nc.dram_tensor("wv", [H, D], MM_DT, kind="ExternalInput").ap()
    aux = nc.dram_tensor("aux", [P, P], MM_DT, kind="ExternalInput").ap()
    out = nc.dram_tensor("out", [NHL, 65, T], F32, kind="ExternalOutput").ap()
    with tile.TileContext(nc) as tc:
        _emit(tc, x, wq, wk, wv, aux, out)
    nc.compile()
    return nc


def _get_nc():
    if "nc" not in _CACHE:
        _CACHE["nc"] = _build()
    return _CACHE["nc"]


def kernel(hidden_states, Wq, bq, Wk, bk, Wv, bv, **_):
    np_dt = np.float32 if MM_DT != BF16 else ml_dtypes.bfloat16
    hidden_states = np.asarray(hidden_states, dtype=np_dt)
    Wq = np.asarray(Wq, dtype=np_dt)
    Wk = np.asarray(Wk, dtype=np_dt)
    Wv = np.asarray(Wv, dtype=np_dt)
    B, S, Hf = hidden_states.shape

    nc = _get_nc()
    in_maps = []
    for k in range(8):
        b, g = k // 2, k % 2
        sl = slice(g * D, (g + 1) * D)
        in_maps.append({
            "x": np.ascontiguousarray(hidden_states[b]),
            "wq": np.ascontiguousarray(Wq[:, sl]),
            "wk": np.ascontiguousarray(Wk[:, sl]),
            "wv": np.ascontiguousarray(Wv[:, sl]),
            "aux": np.eye(P, dtype=np_dt),
        })
    res = run_bass_kernel_spmd(nc, in_maps, core_ids=list(range(8)))

    outf = np.empty((B, S, Hf), dtype=np.float32)
    for k in range(8):
        b, g = k // 2, k % 2
        r = res.results[k]["out"]                  # [8, 65, 2048]
        ctx = r[:, :64, :] / r[:, 64:65, :]        # [8, 64, 2048]
        outf[b, :, g * D:(g + 1) * D] = (
            ctx.transpose(2, 0, 1).reshape(T, D))
    return outf
